# revision 50
# baseline (speedup 1.0000x reference)
"""Trainium2 Bass kernel for nn_MultiHeadSelfAttention_88725434400988.

Self-contained: accepts FULL inputs, shards batch B=256 over 8 NeuronCores
(32 per core), runs one SPMD Bass program, gathers the FULL output.

Per-core schedule (B_CORE=32, S=8, F=32, E=64, A=64, NH=2), v12:
  - fp16 weights/activations, fp32 PSUM accumulation; fp16 output with the
    relu applied on the host (f16-cast and max(x,0) commute exactly).
  - WEIGHT FOLD: the output projection distributes over the attention sum,
      out[q,e] = relu( sum_g (sum_k attn_g[q,k] * vw_g[k,e]) + bias )
    with vw = (Wv @ Wres-half) precomputed PER (key-parity chi, query-parity
    j) on the host as one (65, 256) matrix; the bias rides row 64 (driven by
    a ones-row appended to hsv, halved since both j-slabs add it) and the
    softmax denominators ride column 64 of the attention rhs (ones).  This
    removes the separate v/residual matmuls, any transpose, and the whole
    residual epilogue (PE per attention batch: 1544 cyc vs 3584 baseline).
  - Head: hsT then the wq chunks on the sync queue (8KB-contiguous
    descriptors, no per-tile split); hsv halves + wvres on gpsimd.  PE runs
    a clock-warm burst then the FIRST HALF of the vw projection (padded
    with dummy matmuls to hold the HAM clock) over the DMA head, so wq
    matmuls start ~14us in instead of ~22.
  - Wq stream solo on sync at full rate, then Wk on the gpsimd queue, with
    the wk prefetch GATED on mid-wq progress (tile 22) so it cannot steal
    wq bandwidth; the SECOND HALF of the vw projection fills the
    wq->wk handoff window; psum evac per chunk split across ScalarE
    (jb=0 Copy) and VectorE; qt/kt partition shifts ride the scalar queue
    (kt split scalar+sync); a dep-free warm burst covers the kt shift.
  - Attention per batch b (2-deep exp pipeline): z^T (4 matmuls, keys on
    partitions), one (128,1024) exp on ScalarE -> bf16, 8 avw passes
    lhsT=exp(z)[keys, query-parity half] (full M=128), rhs=[vw|1] (N=65)
    accumulating the key halves: slab[q, g, j, 0:64] is the unnormalized
    output, col 64 the denominator.  reciprocal_approx_fast + one
    broadcast tensor_mul (j=0, both heads) + per-head fused
    scalar_tensor_tensor (mult, add) writes fp16 fo directly; output DMAs
    batched per 8 batches on the idle sync queue.
"""
import numpy as np

B, S, F, E, A, NH = 256, 8, 32, 64, 64, 2
NCORES = 8
BC = B // NCORES            # 32 batches per core
ROWS = BC * S               # 256 projection rows
CD = F * E                  # 2048 contraction dim
ND = A * F * NH             # 4096 projection cols
KTILES = CD // 128          # 16
TTILES = ND // 128          # 32 column tiles per weight
NB = BC * NH                # 64 attention batches per core
WCHUNK = 2                  # weight tiles per DMA chunk / psum group
NCHUNK = TTILES // WCHUNK   # 16 chunks per weight

_NC_CACHE = None


def build_bass():
    import concourse.bacc as bacc
    import concourse.tile as tile
    from concourse import mybir

    f16 = mybir.dt.float16
    bf16 = mybir.dt.bfloat16
    f32 = mybir.dt.float32
    Exp = mybir.ActivationFunctionType.Exp
    Copy = mybir.ActivationFunctionType.Copy
    Add = mybir.AluOpType.add
    Max = mybir.AluOpType.max
    Mult = mybir.AluOpType.mult

    nc = bacc.Bacc("TRN2", target_bir_lowering=False, debug=False)

    hst_d = nc.dram_tensor("hst", [128, KTILES, ROWS], f16, kind="ExternalInput")
    hsv_d = nc.dram_tensor("hsv", [E, NB, 128], f16, kind="ExternalInput")
    wq_d = nc.dram_tensor("wq", [128, TTILES, KTILES * 128], f16,
                          kind="ExternalInput")
    wk_d = nc.dram_tensor("wk", [128, TTILES, KTILES * 128], f16,
                          kind="ExternalInput")
    wvres_d = nc.dram_tensor("wvres", [E + 1, 256], f16, kind="ExternalInput")
    out_d = nc.dram_tensor("out", [128, BC * 128], f16, kind="ExternalOutput")

    with tile.TileContext(nc) as tc:
        from contextlib import ExitStack
        with ExitStack() as ctx:
            singles = ctx.enter_context(tc.tile_pool(name="singles", bufs=1))

            # ---- constants / persistent tiles ----
            ones_bf = singles.tile([128, A], bf16)
            nc.vector.memset(ones_bf, 1.0)
            warm_t = singles.tile([128, 256], f16)
            nc.vector.memset(warm_t, 0.25)
            dummy_e = singles.tile([128, 8], bf16)

            hsT = singles.tile([128, KTILES, ROWS], f16)
            hsv = singles.tile([E + 1, NB, 128], f16)   # row 64 = ones (bias)
            nc.vector.memset(hsv[E:E + 1, :, :], 1.0)
            wvres_sb = singles.tile([E + 1, 256], f16)

            qt = singles.tile([64, 2, BC, NH, 128], f16)
            kt_ = singles.tile([64, 2, BC, NH, 128], f16)
            # vw[sigma, bn, chi(=kk), j, 0:64]=folded V*Wres rows; col 64=1
            vw_all = singles.tile([128, NB, 2, 2, 65], bf16)
            nc.vector.memset(vw_all[:, :, :, :, 64:65], 1.0)

            # ---- input DMAs: hsT FIRST on sync (ahead of the wq chunks it
            # gates anyway), hsv halves + wvres on the (otherwise idle until
            # the gate) gpsimd queue ----
            nc.sync.dma_start(hsT[:, :, :], hst_d[:])
            nc.gpsimd.dma_start(wvres_sb[:, :], wvres_d[:])
            nc.gpsimd.dma_start(hsv[0:E, 0:NB // 2, :], hsv_d[:, 0:NB // 2, :])
            nc.gpsimd.dma_start(hsv[0:E, NB // 2:NB, :],
                                hsv_d[:, NB // 2:NB, :])

            # pre-load the Exp act table during the head
            nc.scalar.activation(dummy_e[:, :], warm_t[:, 0:8], Exp)

            # ---- PE clock warm-up on memset data while the head DMAs run --
            with tc.tile_pool(name="pwarm", bufs=1, space="PSUM") as pw_pool:
                pw = pw_pool.tile([A, 256], f32)
                for wi in range(30):
                    nc.tensor.matmul(
                        pw[:, :], lhsT=ones_bf[:, :], rhs=warm_t[:, :],
                        start=(wi == 0), stop=(wi == 29))

            def emit_vproj(pair, vps_pool):
                vt = vps_pool.tile([128, 2, 256], f32, name="vt", tag="vt")
                for jb in range(2):
                    nc.tensor.matmul(
                        vt[:, jb, :],
                        lhsT=hsv[:, pair * 2 + jb, :],
                        rhs=wvres_sb[:, :],
                        start=True, stop=True)
                for jb in range(2):
                    bn = pair * 2 + jb
                    dst = vw_all[:, bn, :, :, 0:64]
                    sv = vt[:, jb, :].rearrange("p (c j e) -> p c j e", c=2,
                                                j=2)
                    if jb == 0:
                        nc.scalar.activation(dst, sv, Copy)
                    else:
                        nc.vector.tensor_copy(dst, sv)

            # ---- vw projection, first half: covers the weight DMA head ----
            vp = NB // 4
            with tc.tile_pool(name="vps1", bufs=4, space="PSUM") as vps1, \
                 tc.tile_pool(name="padh", bufs=1, space="PSUM") as padh_pool:
                padh = padh_pool.tile([A, ROWS], f32)
                for pair in range(NB // 4):
                    emit_vproj(pair, vps1)
                    for wi in range(2):
                        nc.tensor.matmul(
                            padh[:, :], lhsT=ones_bf[:, :], rhs=warm_t[:, :],
                            start=(wi == 0), stop=(wi == 1))

            # ---- sequential Wq then Wk streams (baseline-proven DMA plan:
            # wq runs solo on the sync queue at full rate; wk prefetch on
            # the gpsimd queue is GATED on mid-wq progress so it cannot
            # steal head bandwidth) ----
            with tc.tile_pool(name="wtq", bufs=3) as w_pool_q, \
                 tc.tile_pool(name="wtk", bufs=3) as w_pool_k, \
                 tc.tile_pool(name="stages", bufs=2) as st_pool, \
                 tc.tile_pool(name="vps2", bufs=2, space="PSUM") as vps2, \
                 tc.tile_pool(name="pp", bufs=3, space="PSUM") as pp_pool:

                stage_q = st_pool.tile([128, BC, NH, 128], f16, name="sq",
                                       tag="st")
                stage_k = st_pool.tile([128, BC, NH, 128], f16, name="sk",
                                       tag="st")
                gate_t = singles.tile([1, 4], f16)

                def proj_stream(w_d, dest, dma_eng, stage, w_pool,
                                shift_engs, split_tiles=False):
                    for tg in range(NCHUNK):
                        wt = w_pool.tile([128, WCHUNK, KTILES, 128], f16,
                                         name="wt", tag="wt")
                        if split_tiles:
                            for ti in range(WCHUNK):
                                dma_eng.dma_start(
                                    wt[:, ti, :, :],
                                    w_d[:, tg * WCHUNK + ti, :]
                                    .rearrange("p (kt c) -> p kt c", c=128))
                        else:
                            dma_eng.dma_start(
                                wt[:, :, :, :],
                                w_d[:, tg * WCHUNK:(tg + 1) * WCHUNK, :]
                                .rearrange("p t (kt c) -> p t kt c", c=128))
                        pp = pp_pool.tile([128, WCHUNK, ROWS], f32,
                                          name="pp", tag="pp")
                        for ti in range(WCHUNK):
                            for kt in range(KTILES):
                                nc.tensor.matmul(
                                    pp[:, ti, :],
                                    lhsT=wt[:, ti, kt, :],
                                    rhs=hsT[:, kt, :],
                                    start=(kt == 0),
                                    stop=(kt == KTILES - 1))
                        src = pp.rearrange(
                            "p ti (b n sp) -> p (b n) ti sp", n=NH, sp=4)
                        t0 = tg * WCHUNK
                        dv = dest[:, 0, :, :, :].rearrange(
                            "p b n (f sp) -> p (b n) f sp", sp=4)
                        sv = stage[:, :, :, :].rearrange(
                            "p b n (f sp) -> p (b n) f sp", sp=4)
                        nc.vector.tensor_copy(
                            dv[:, :, t0:t0 + WCHUNK, :], src[0:64])
                        nc.vector.tensor_copy(
                            sv[64:128, :, t0:t0 + WCHUNK, :], src[64:128])
                    # partition shift 64..127 -> 0..63 (16KB runs)
                    if len(shift_engs) == 1:
                        shift_engs[0].dma_start(
                            dest[:, 1, :, :, :], stage[64:128, :, :, :])
                    else:
                        h = BC // 2
                        shift_engs[0].dma_start(
                            dest[:, 1, 0:h, :, :], stage[64:128, 0:h, :, :])
                        shift_engs[1].dma_start(
                            dest[:, 1, h:BC, :, :], stage[64:128, h:BC, :, :])

                proj_stream(wq_d, qt, nc.sync, stage_q, w_pool_q,
                            [nc.scalar])
                # gate: wk's first chunk DMA (gpsimd) depends on this tiny
                # copy of a mid-wq stage region, keeping the wk prefetch
                # off the bandwidth-critical wq head
                nc.gpsimd.tensor_copy(
                    gate_t[:, :], stage_q[64:65, 0, 0, 88:92])
                # vw projection second half fills the wq->wk handoff PE gap
                # (wk DMA ramps underneath it)
                while vp < NB // 2:
                    emit_vproj(vp, vps2)
                    vp += 1
                proj_stream(wk_d, kt_, nc.gpsimd, stage_k, w_pool_k,
                            [nc.scalar, nc.sync])

                # warm burst keeps the clock at k=8 across the kt shift
                with tc.tile_pool(name="pwarm2", bufs=1,
                                  space="PSUM") as pw_pool2:
                    pw2 = pw_pool2.tile([A, ROWS], f32)
                    for wi in range(14):
                        nc.tensor.matmul(
                            pw2[:, :], lhsT=ones_bf[:, :], rhs=hsT[:, 0, :],
                            start=(wi == 0), stop=(wi == 13))

            # ---- attention: z^T matmuls + exp + folded AVW passes ----
            with tc.tile_pool(name="zps", bufs=2, space="PSUM") as z_pool, \
                 tc.tile_pool(name="pad", bufs=1, space="PSUM") as pad_pool, \
                 tc.tile_pool(name="aps", bufs=2, space="PSUM") as a_pool, \
                 tc.tile_pool(name="expz", bufs=3) as e_pool, \
                 tc.tile_pool(name="recs", bufs=2) as rec_pool, \
                 tc.tile_pool(name="tsum", bufs=2) as ts_pool, \
                 tc.tile_pool(name="fo", bufs=2) as f_pool:

                def emit_zt_exp(b):
                    zt = z_pool.tile([128, 4, 256], f32, name="zt", tag="zt")
                    for nh in range(NH):
                        for h in range(2):
                            nc.tensor.matmul(
                                zt[:, nh * 2 + h, :],
                                lhsT=kt_[:, h, b, nh, :],
                                rhs=qt[:, :, b, nh, :],
                                start=True, stop=True)
                    ez = e_pool.tile([128, 4, 256], bf16, name="ez", tag="ez")
                    nc.scalar.activation(
                        ez[:, :, :].rearrange("p a b -> p (a b)"),
                        zt[:, :, :].rearrange("p a b -> p (a b)"), Exp)
                    return ez

                fo = None
                # dep-free PE padding keeps per-window utilization above the
                # HAM k=4 throttle threshold (the phase is ScalarE-paced, so
                # the padding is hidden; losing k=8 would make the PE the
                # pacer at half clock instead)
                pad = pad_pool.tile([A, ROWS], f32)
                ezq = [emit_zt_exp(0), emit_zt_exp(1)]
                for b in range(BC):
                    cur = ezq[b % 2]
                    if b + 2 < BC:
                        ezq[b % 2] = emit_zt_exp(b + 2)
                    # 8 avw passes: queries (parity j) on M, [vw|1] on N
                    avw = a_pool.tile([128, 2, 2, 65], f32, name="avw",
                                      tag="avw")
                    for g in range(NH):
                        for j in range(2):
                            for kk in range(2):
                                nc.tensor.matmul(
                                    avw[:, g, j, :],
                                    lhsT=cur[:, g * 2 + kk,
                                             j * 128:(j + 1) * 128],
                                    rhs=vw_all[:, b * NH + g, kk, j, :],
                                    start=(kk == 0), stop=(kk == 1))
                    rec = rec_pool.tile([128, 2, 2, 1], f32, name="rec",
                                        tag="rec")
                    nc.vector.reciprocal_approx_fast(
                        rec[:, :, :, :].rearrange("p a b c -> p (a b) c"),
                        avw[:, :, :, 64:65].rearrange("p a b c -> p (a b) c"))
                    # per g: (slab_j0 * rec_j0) + (slab_j1 * rec_j1); bias is
                    # folded into the slabs (hsv ones row); relu runs on the
                    # host (exact: f16 cast then max commute for relu)
                    tsum = ts_pool.tile([128, 2, 64], f32, name="tsum",
                                        tag="tsum")
                    if b % 8 == 0:
                        fo = f_pool.tile([128, 8, 2, 64], f16, name="fo",
                                         tag="fo")
                    nc.vector.tensor_mul(
                        tsum[:, :, :], avw[:, :, 0, 0:64],
                        rec[:, :, 0, :].to_broadcast((128, 2, 64)))
                    for g in range(NH):
                        nc.vector.scalar_tensor_tensor(
                            fo[:, b % 8, g, :], avw[:, g, 1, 0:64],
                            rec[:, g, 1, :], tsum[:, g, :],
                            Mult, Add)
                    if b == 27:
                        # drain the first half of the last group early
                        nc.sync.dma_start(
                            out_d[:, 24 * 128:28 * 128],
                            fo[:, 0:4, :, :].rearrange("p a g e -> p (a g e)"))
                    elif b == 31:
                        nc.sync.dma_start(
                            out_d[:, 28 * 128:32 * 128],
                            fo[:, 4:8, :, :].rearrange("p a g e -> p (a g e)"))
                    elif b % 8 == 7:
                        nc.sync.dma_start(
                            out_d[:, (b - 7) * 128:(b + 1) * 128],
                            fo[:, :, :, :].rearrange("p a g e -> p (a g e)"))
    nc.compile()
    return nc


def _get_nc():
    global _NC_CACHE
    if _NC_CACHE is None:
        _NC_CACHE = build_bass()
    return _NC_CACHE


def _prep_weight(W):
    # (CD, ND) -> (128, TTILES, KTILES*128): [p, t, kt*128+j] = W[kt*128+p, t*128+j]
    return np.ascontiguousarray(
        W.astype(np.float16).reshape(KTILES, 128, TTILES, 128)
        .transpose(1, 2, 0, 3).reshape(128, TTILES, KTILES * 128))


def make_in_maps(Hs, Wq, Wk, Wv, Wres_w, Wres_b):
    wq16 = _prep_weight(Wq)
    wk16 = _prep_weight(Wk)
    # folded output weights: wvres[E', chi, j, e] = sum_i Wv[E', chi*64+i]
    #   * Wres[j*64+i, e];  row E (ones-driven) carries bias/2 per j-slab
    wv = Wv.astype(np.float32).reshape(E, 2, A)            # (E', chi, i)
    wr = Wres_w.astype(np.float32).reshape(2, A, E)        # (j, i, e)
    wvres = np.einsum('xci,jie->xcje', wv, wr)             # (E', 2, 2, E)
    wvres_aug = np.concatenate(
        [wvres, np.broadcast_to(Wres_b.astype(np.float32) * 0.5,
                                (1, 2, 2, E))], axis=0)
    wvres16 = np.ascontiguousarray(
        wvres_aug.reshape(E + 1, 256).astype(np.float16))
    hs16 = Hs.astype(np.float16)
    maps = []
    for c in range(NCORES):
        sh = hs16[c * BC:(c + 1) * BC]                      # (BC, S, CD)
        hs2d = sh.reshape(ROWS, CD)
        hst = np.ascontiguousarray(
            hs2d.reshape(ROWS, KTILES, 128).transpose(2, 1, 0))
        # v rows in sigma order (f*4+sp):
        # hsv[e, q, f*4+sp] = Hs[b, nh*4+sp, f, e]; bn = 2q+pi = b*NH+nh
        arr = sh.reshape(NB, 4, F, E).transpose(0, 2, 1, 3).reshape(NB, 128, E)
        hsv = np.ascontiguousarray(arr.transpose(2, 0, 1))  # (E, NB, sigma)
        maps.append({
            "hst": hst, "hsv": hsv,
            "wq": wq16, "wk": wk16, "wvres": wvres16,
        })
    return maps


def _unpack_out(o):
    # o: (128, BC*128) rows q=(t, sp), cols (b, g, e) -> (BC, S, F*E)
    # relu runs here (host): f16-cast and max(x,0) commute exactly
    o = np.maximum(o.astype(np.float32), 0.0)
    o = o.reshape(F, 4, BC, 2, E)                  # (t, sp, b, g, e)
    return np.ascontiguousarray(
        o.transpose(2, 3, 1, 0, 4)).reshape(BC, S, F * E)


def kernel(Hs, Wq, Wk, Wv, Wres_w, Wres_b):
    from concourse.bass_utils import run_bass_kernel_spmd
    nc = _get_nc()
    in_maps = make_in_maps(Hs, Wq, Wk, Wv, Wres_w, Wres_b)
    res = run_bass_kernel_spmd(nc, in_maps, list(range(NCORES)))
    out = np.concatenate(
        [_unpack_out(np.asarray(res.results[c]["out"]))
         for c in range(NCORES)], axis=0)
    return out.astype(np.float32)


if __name__ == "__main__":
    nc = build_bass()
    print("built OK; instructions:",
          sum(len(bb.instructions) for fn in nc.m.functions
              for bb in fn.blocks))
